# revision 1
# baseline (speedup 1.0000x reference)
"""Trainium2 Bass kernel: cosine-similarity softmin retrieval (DSDM).

reference:  qn = q/||q||; an = a/||a||; sims = qn @ an^T            [B, N]
            w = softmax(10*sims) over N  (softmin of (1-sims)/0.1)
            out = (w @ A)                                           [B, D]

Strategy (8 NeuronCores, flash-attention-style split over N):
  - addresses [200000, 512] sharded row-wise, 25000 rows/core.
  - each core streams its shard once in 128-row tiles (bf16 on-chip, cast
    during the load DMA):
      * row norms ss = sum(a^2) on DVE (affine_mul_reduce)
      * 10/||a|| = exp(-0.5*ln(ss + eps) + ln10) on ACT (one table set)
      * A^T chunks via HWDGE xbar DMA-transpose (bf16, SBUF->SBUF) -- frees
        the PE from 2 of its 3 passes over A and avoids a PSUM->SBUF copy
      * s_raw^T [128j, 64b] = A_chunk @ qn^T via 4 PSUM-accumulated matmuls
      * w^T = Exp(s_raw^T * (10/||a||) - 10) on ACT (fixed shift: cos<=1,
        so logit-10 <= 0; no running max needed)
      * acc [64, 512] += w^T.T @ A in PSUM across all tiles
      * wsum [128, 64] += w^T on GPSIMD; ones-matmul partition-reduce at end
  - host: out = sum_c acc_c / sum_c l_c   (gather/unshard + tiny divide)

Padding: per-core row count 25000 = 195*128 + 40; the last tile's 88 pad
rows are zeroed and get exp bias -40 (weight ~4e-18, exactly negligible).
"""

import math
import os
from collections import OrderedDict

import numpy as np

import concourse.bass as bass
import concourse.tile as tile
from concourse import bacc, mybir
from concourse.bass_utils import run_bass_kernel_spmd
from concourse.masks import make_identity

DT = mybir.dt
AF = mybir.ActivationFunctionType
ALU = mybir.AluOpType

B = 64
D = 512
N_FULL = 200000
NCORES = 8
NPC = N_FULL // NCORES  # 25000
P = 128
G = 4  # tiles per DMA slab
LN10 = math.log(10.0)

# "pe" or "dma": how A^T chunks are produced
TRANSPOSE_MODE = os.environ.get("KERNEL_TRANSPOSE", "pe")
NORMS_MODE = os.environ.get("KERNEL_NORMS", "mixed")
NORM_DVE_OF8 = int(os.environ.get("KERNEL_NORM_DVE_OF8", "4"))  # tiles/8 on DVE
WSUM_MODE = os.environ.get("KERNEL_WSUM", "gpsimd")
SIMS_MODE = os.environ.get("KERNEL_SIMS", "quad")

LAST_RESULTS = None  # test harness reads exec_time_ns from here


def _patch_act_tables():
    """Prefer the combined natural_log_exp set so Ln/Exp/Square/Copy share
    one ACT table load instead of thrashing 2 loads per slab (~2.7us each)."""
    if getattr(bacc.get_activation_tables, "_patched", False):
        return
    orig = bacc.get_activation_tables

    keep = {AF.Ln, AF.Exp, AF.Square}

    def patched(arch):
        tabs = orig(arch)
        out = OrderedDict()
        for k, fns in tabs.items():
            if k == "natural_log_exp_and_others":
                out[k] = fns
            else:
                out[k] = {f for f in fns if f not in keep}
        return out

    patched._patched = True
    bacc.get_activation_tables = patched


def _build(npc=NPC):
    _patch_act_tables()
    ntiles = (npc + P - 1) // P
    G = max(g for g in range(1, 17) if ntiles % g == 0)  # tiles per slab
    nslabs = ntiles // G
    real_last = npc - (ntiles - 1) * P  # rows in final tile

    nc = bacc.Bacc("TRN2")
    q_d = nc.dram_tensor("query", [B, D], DT.float32, kind="ExternalInput")
    a_d = nc.dram_tensor("addresses", [npc, D], DT.float32, kind="ExternalInput")
    acc_d = nc.dram_tensor("acc", [B, D], DT.float32, kind="ExternalOutput")
    lsum_d = nc.dram_tensor("lsum", [B, 1], DT.float32, kind="ExternalOutput")

    with tile.TileContext(nc) as tc:
        with (
            tc.tile_pool(name="const", bufs=1) as const,
            tc.tile_pool(name="slab", bufs=4) as slab_pool,
            tc.tile_pool(name="at", bufs=8) as at_pool,
            tc.tile_pool(name="wt", bufs=4) as wt_pool,
            tc.tile_pool(name="small", bufs=4) as small,
            tc.tile_pool(name="ps_at", bufs=2, space="PSUM") as ps_at,
            tc.tile_pool(name="ps_s", bufs=2, space="PSUM") as ps_s,
            tc.tile_pool(name="ps_wt", bufs=2, space="PSUM") as ps_wt,
            tc.tile_pool(name="ps_one", bufs=1, space="PSUM") as ps_one,
            tc.tile_pool(name="ps_acc", bufs=1, space="PSUM") as ps_acc,
            tc.tile_pool(name="dram", bufs=1, space="DRAM") as dram_pool,
        ):
            ident = const.tile([P, P], DT.bfloat16)
            make_identity(nc, ident)
            bias_main = const.tile([P, 1], DT.float32)
            nc.vector.memset(bias_main, -10.0)
            bias_last = const.tile([P, 1], DT.float32)
            nc.vector.memset(bias_last, -40.0)
            if real_last > 0:
                nc.vector.memset(bias_last[:real_last], -10.0)
            ones = const.tile([P, 1], DT.float32)
            nc.vector.memset(ones, 1.0)
            eps12 = const.tile([P, 1], DT.float32)
            nc.vector.memset(eps12, 1e-12)
            ln10b = const.tile([P, 1], DT.float32)
            nc.vector.memset(ln10b, LN10)
            wsum = const.tile([P, B], DT.float32)
            nc.vector.memset(wsum, 0.0)
            wsum4 = const.tile([P, 4, B], DT.float32)
            nc.vector.memset(wsum4, 0.0)
            identf = const.tile([P, P], DT.float32)
            make_identity(nc, identf)

            # ---- query preprocessing: qn^T bf16 chunks [128d, 4c, 64b] ----
            q_sb = const.tile([B, D], DT.float32)
            nc.sync.dma_start(out=q_sb, in_=q_d[:, :])
            qsq = const.tile([B, D], DT.float32)
            ssq = const.tile([B, 1], DT.float32)
            nc.scalar.activation(qsq, q_sb, AF.Square, accum_out=ssq)
            lnq = const.tile([B, 1], DT.float32)
            nc.scalar.activation(lnq, ssq, AF.Ln, bias=eps12[:B])
            invq = const.tile([B, 1], DT.float32)
            nc.scalar.activation(invq, lnq, AF.Exp, scale=-0.5)
            qn = const.tile([B, D], DT.bfloat16)
            nc.vector.tensor_scalar_mul(out=qn, in0=q_sb, scalar1=invq)
            qnT = const.tile([P, 4, B], DT.bfloat16)
            for c in range(4):
                qt_ps = ps_one.tile([P, B], DT.bfloat16, tag="onebank")
                nc.tensor.transpose(qt_ps, qn[:, c * P:(c + 1) * P], ident[:B, :B])
                nc.scalar.copy(qnT[:, c, :], qt_ps)

            # ---- main streaming loop ----
            acc_ps = ps_acc.tile([B, D], DT.float32)
            nquads = ntiles // 4
            assert SIMS_MODE == "tile" or nquads * 4 == ntiles
            scr = dram_pool.tile([1, ntiles * P], DT.float32)
            slab_tiles = {}
            slab_inv = {}

            def ensure_slab(g):
                if g in slab_tiles:
                    return slab_tiles[g]
                a_sl = slab_pool.tile([P, G, D], DT.bfloat16)
                last_slab = g == nslabs - 1
                if not last_slab or real_last == P:
                    nc.gpsimd.dma_start(
                        out=a_sl,
                        in_=a_d[g * G * P:(g + 1) * G * P, :].rearrange(
                            "(t p) d -> p t d", p=P))
                else:
                    for t in range(G - 1):
                        r0 = (g * G + t) * P
                        nc.gpsimd.dma_start(out=a_sl[:, t, :], in_=a_d[r0:r0 + P, :])
                    nc.gpsimd.memset(a_sl[:, G - 1, :], 0)
                    nc.gpsimd.dma_start(
                        out=a_sl[:real_last, G - 1, :],
                        in_=a_d[(ntiles - 1) * P:npc, :])
                slab_tiles[g] = a_sl
                # norms for the slab + 10/||a|| + transposed flat copy to DRAM
                ss = small.tile([P, G], DT.float32, tag="ss")
                for t in range(G):
                    gt0 = g * G + t
                    sq = small.tile([P, D], DT.bfloat16, tag="sq")
                    if (gt0 % 8) < NORM_DVE_OF8:
                        nc.vector.affine_mul_reduce(
                            out=sq, accum_out=ss[:, t:t + 1],
                            in0=a_sl[:, t, :], in1=a_sl[:, t, :], scale=1.0,
                            bias=0.0)
                    else:
                        nc.scalar.activation(sq, a_sl[:, t, :], AF.Square,
                                             accum_out=ss[:, t:t + 1])
                lns = small.tile([P, G], DT.float32, tag="lns")
                nc.scalar.activation(lns, ss, AF.Ln, bias=eps12)
                inv = small.tile([P, G], DT.float32, tag="inv")
                nc.scalar.activation(inv, lns, AF.Exp, scale=-0.5, bias=ln10b)
                slab_inv[g] = inv
                if SIMS_MODE == "quad":
                    ivt_ps = ps_one.tile([G, P], DT.float32, tag="onebank")
                    nc.tensor.transpose(ivt_ps, inv, identf)
                    ivt = small.tile([G, P], DT.float32, tag="ivt_sb")
                    nc.vector.tensor_copy(ivt, ivt_ps)
                    nc.sync.dma_start(out=a_scr_view(g), in_=ivt)
                return a_sl

            def a_scr_view(g):
                return bass.AP(
                    tensor=scr.tensor, offset=scr.offset + g * G * P,
                    ap=[[P, G], [1, P]])

            def a_tile(gt):
                g, t = divmod(gt, G)
                return ensure_slab(g)[:, t, :]

            if SIMS_MODE == "quad":
                pending = None  # (q, w_q) awaiting back stage

                def stage_front(q):
                    at_tiles = []
                    for t in range(4):
                        gt = 4 * q + t
                        a_t = a_tile(gt)
                        at_sb = at_pool.tile([P, 4, P], DT.bfloat16)
                        at_ps = ps_at.tile([P, 4, P], DT.bfloat16)
                        for c in range(4):
                            nc.tensor.transpose(
                                at_ps[:, c, :], a_t[:, c * P:(c + 1) * P], ident)
                        nc.vector.tensor_copy(at_sb, at_ps)
                        at_tiles.append(at_sb)
                    inv_bc = wt_pool.tile([B, 4 * P], DT.float32, tag="inv_bc")
                    nc.gpsimd.dma_start(
                        out=inv_bc,
                        in_=bass.AP(tensor=scr.tensor,
                                    offset=scr.offset + q * 4 * P,
                                    ap=[[0, B], [1, 4 * P]]))
                    s_ps = ps_s.tile([B, 4 * P], DT.float32, tag="s")
                    for t in range(4):
                        for c in range(4):
                            nc.tensor.matmul(
                                s_ps[:, t * P:(t + 1) * P],
                                lhsT=qnT[:, c, :], rhs=at_tiles[t][:, c, :],
                                start=(c == 0), stop=(c == 3))
                    s_sc = wt_pool.tile([B, 4 * P], DT.float32, tag="s_sc")
                    nc.vector.tensor_mul(s_sc, s_ps, inv_bc)
                    w_q = wt_pool.tile([B, 4 * P], DT.bfloat16, tag="w_q")
                    nc.scalar.activation(w_q, s_sc, AF.Exp, bias=bias_main[:B])
                    return w_q

                def stage_back(q, w_q):
                    wt_ps = ps_wt.tile([P, 4, B], DT.bfloat16)
                    for t in range(4):
                        nc.tensor.transpose(
                            wt_ps[:, t, :], w_q[:, t * P:(t + 1) * P],
                            ident[:B, :B])
                    wt_sb = wt_pool.tile([P, 4, B], DT.bfloat16, tag="wt_sb")
                    nc.vector.tensor_copy(wt_sb, wt_ps)
                    for t in range(4):
                        gt = 4 * q + t
                        nc.tensor.matmul(
                            acc_ps, lhsT=wt_sb[:, t, :], rhs=a_tile(gt),
                            start=(gt == 0), stop=(gt == ntiles - 1))
                    nc.gpsimd.tensor_add(wsum4, wsum4, wt_sb)

                for q in range(nquads):
                    w_q = stage_front(q)
                    if pending is not None:
                        stage_back(*pending)
                    pending = (q, w_q)
                if pending is not None:
                    stage_back(*pending)
            else:
                for gt in range(ntiles):
                    g, t = divmod(gt, G)
                    a_sl = ensure_slab(g)
                    at_sb = at_pool.tile([P, 4, P], DT.bfloat16)
                    at_ps = ps_at.tile([P, 4, P], DT.bfloat16)
                    for c in range(4):
                        nc.tensor.transpose(
                            at_ps[:, c, :], a_sl[:, t, c * P:(c + 1) * P], ident)
                    nc.vector.tensor_copy(at_sb, at_ps)
                    s_ps = ps_s.tile([P, B], DT.float32, tag="s")
                    for c in range(4):
                        nc.tensor.matmul(
                            s_ps, lhsT=at_sb[:, c, :], rhs=qnT[:, c, :],
                            start=(c == 0), stop=(c == 3))
                    wt = wt_pool.tile([P, B], DT.bfloat16, tag="wt")
                    inv = slab_inv[g]
                    nc.scalar.activation(
                        wt, s_ps, AF.Exp,
                        bias=bias_last if gt == ntiles - 1 else bias_main,
                        scale=inv[:, t:t + 1])
                    nc.tensor.matmul(
                        acc_ps, lhsT=wt, rhs=a_sl[:, t, :],
                        start=(gt == 0), stop=(gt == ntiles - 1))
                    nc.gpsimd.tensor_add(wsum, wsum, wt)

            # ---- epilogue: normalizer + writeback ----
            l_ps = ps_one.tile([B, 1], DT.float32, tag="onebank")
            if SIMS_MODE == "quad":
                for t in range(4):
                    nc.tensor.matmul(l_ps, lhsT=wsum4[:, t, :], rhs=ones,
                                     start=(t == 0), stop=(t == 3))
            else:
                nc.tensor.matmul(l_ps, lhsT=wsum, rhs=ones)
            acc_sb = const.tile([B, D], DT.float32)
            nc.scalar.copy(acc_sb, acc_ps)
            l_sb = const.tile([B, 1], DT.float32)
            nc.vector.tensor_copy(l_sb, l_ps)
            nc.sync.dma_start(out=acc_d[:, :], in_=acc_sb)
            nc.sync.dma_start(out=lsum_d[:, :], in_=l_sb)

    nc.finalize()
    return nc


_NC_CACHE = {}


def _get_nc(npc=NPC):
    if npc not in _NC_CACHE:
        _NC_CACHE[npc] = _build(npc)
    return _NC_CACHE[npc]


def kernel(query, addresses):
    global LAST_RESULTS
    query = np.ascontiguousarray(np.asarray(query), dtype=np.float32)
    addresses = np.ascontiguousarray(np.asarray(addresses), dtype=np.float32)
    n = addresses.shape[0]
    npc = n // NCORES
    assert npc * NCORES == n
    nc = _get_nc(npc)
    in_maps = [
        {"query": query, "addresses": addresses[c * npc:(c + 1) * npc]}
        for c in range(NCORES)
    ]
    res = run_bass_kernel_spmd(nc, in_maps, core_ids=list(range(NCORES)))
    LAST_RESULTS = res
    acc = np.zeros((B, D), np.float64)
    l = np.zeros((B, 1), np.float64)
    ntiles = (npc + P - 1) // P
    n_pad = ntiles * P - npc  # zero rows in the padded last tile
    for r in res.results:
        acc += r["acc"].astype(np.float64)
        l += r["lsum"].astype(np.float64)
        if SIMS_MODE == "quad" and n_pad:
            # each pad row contributes exactly exp(0*scale - 10)
            l -= n_pad * math.exp(-10.0)
    return (acc / l).astype(np.float32)



# revision 10
# speedup vs baseline: 1.0747x; 1.0747x over previous
"""Trainium2 Bass kernel: cosine-similarity softmin retrieval (DSDM).

reference:  qn = q/||q||; an = a/||a||; sims = qn @ an^T            [B, N]
            w = softmax(10*sims) over N  (softmin of (1-sims)/0.1)
            out = (w @ A)                                           [B, D]

v2 strategy (8 NeuronCores, flash-attention-style split over N):
  - addresses [200000, 512] sharded row-wise, 25000 rows/core.
  - per core the shard streams once in 512-row "quads" (49 of them):
      * one SWDGE cast-DMA per quad with row-permuted layout
        (p t) d -> p t d: partition p holds rows 4p..4p+3 => 8KB
        contiguous HBM descriptors.  The row permutation cancels
        between the sims and retrieval matmuls, so no unpermute needed.
      * row norms ss = sum(a^2) split across DVE/ACT per env pattern
      * inv = 10/||a|| = exp(-0.5*ln(ss+eps) + ln10) on ACT (one table)
      * diag(inv) built per 128-row tile on DVE (ident * inv)
      * A^T chunks produced by PLAIN matmuls lhsT=A_chunk, rhs=diag
        -> PSUM holds A^T pre-scaled by 10/||a||; plain matmul (not
        transpose-mode) keeps the HAM clock gate warm.
      * PSUM->SBUF copies of A^T split DVE/ACT per env pattern
      * sims: 4 matmuls N=512 per quad (lhsT=qn^T chunks stationary)
      * w = Exp(s - 10) on ACT, one [64, 512] activation per quad
        (fixed shift: cos<=1 so logit-10 <= 0; no running max needed)
      * w^T via 4 plain matmuls rhs=I64; acc [64,512] += w^T.T @ A in
        PSUM across all tiles; lsum accumulated either by N=1 matmuls
        (reusing the retrieval stationary) or GPSIMD adds.
  - host: out = sum_c acc_c / sum_c l_c   (gather/unshard + tiny divide)

Padding: per-core 25000 rows = 48 full quads + 424 rows (partitions
0..105 of quad 48); partitions 106..127 are zeroed and contribute
exactly exp(-10) each to lsum, subtracted on the host.
"""

import math
import os
from collections import OrderedDict

import numpy as np

import concourse.bass as bass
import concourse.tile as tile
from concourse import bacc, mybir
from concourse.bass_utils import run_bass_kernel_spmd
from concourse.masks import make_identity

DT = mybir.dt
AF = mybir.ActivationFunctionType

B = 64
D = 512
N_FULL = 200000
NCORES = 8
NPC = N_FULL // NCORES  # 25000
P = 128
QROWS = 4 * P  # rows per quad
LN10 = math.log(10.0)

# engine assignment knobs (v=DVE, a=ACT, g=GPSIMD), one char per tile-in-quad
NORM_PAT = os.environ.get("KERNEL_NORM_PAT", "vvvv")
ATCOPY_PAT = os.environ.get("KERNEL_ATCOPY_PAT", "aaaa")
WSUM_MODE = os.environ.get("KERNEL_WSUM", "pe")  # "pe" or "gpsimd"
WTCOPY_ENG = os.environ.get("KERNEL_WTCOPY", "v")  # v|a (gpsimd has no PSUM access)
ABUFS = int(os.environ.get("KERNEL_ABUFS", "6"))

LAST_RESULTS = None  # test harness reads exec_time_ns from here


def _patch_act_tables():
    """Prefer the combined natural_log_exp set so Ln/Exp/Square/Copy share
    one ACT table load instead of thrashing 2 loads per quad (~2.7us each)."""
    if getattr(bacc.get_activation_tables, "_patched", False):
        return
    orig = bacc.get_activation_tables

    keep = {AF.Ln, AF.Exp, AF.Square, AF.Copy}

    def patched(arch):
        tabs = orig(arch)
        out = OrderedDict()
        for k, fns in tabs.items():
            if k == "natural_log_exp_and_others":
                out[k] = fns
            else:
                out[k] = {f for f in fns if f not in keep}
        return out

    patched._patched = True
    bacc.get_activation_tables = patched


def _build(npc=NPC):
    _patch_act_tables()
    assert npc % 4 == 0
    nquads = (npc + QROWS - 1) // QROWS
    ntiles = 4 * nquads
    # real rows in the last quad; they occupy partitions [0, p_real) slots 0..3
    r_last = npc - (nquads - 1) * QROWS
    assert r_last % 4 == 0
    p_real = r_last // 4  # 106 for npc=25000

    nc = bacc.Bacc("TRN2")
    q_d = nc.dram_tensor("query", [B, D], DT.float32, kind="ExternalInput")
    a_d = nc.dram_tensor("addresses", [npc, D], DT.float32, kind="ExternalInput")
    acc_d = nc.dram_tensor("acc", [B, D], DT.float32, kind="ExternalOutput")
    lsum_d = nc.dram_tensor("lsum", [B, 1], DT.float32, kind="ExternalOutput")

    with tile.TileContext(nc) as tc:
        with (
            tc.tile_pool(name="const", bufs=1) as const,
            tc.tile_pool(name="slab", bufs=ABUFS) as slab_pool,
            tc.tile_pool(name="at", bufs=3) as at_pool,
            tc.tile_pool(name="wq", bufs=3) as wq_pool,
            tc.tile_pool(name="wt", bufs=3) as wt_pool,
            tc.tile_pool(name="small", bufs=4) as small,
            tc.tile_pool(name="ps_at", bufs=2, space="PSUM") as ps_at,
            tc.tile_pool(name="ps_s", bufs=2, space="PSUM") as ps_s,
            tc.tile_pool(name="ps_wt", bufs=2, space="PSUM") as ps_wt,
            tc.tile_pool(name="ps_acc", bufs=1, space="PSUM") as ps_acc,
            tc.tile_pool(name="ps_l", bufs=1, space="PSUM") as ps_l,
        ):
            ident = const.tile([P, P], DT.bfloat16)
            make_identity(nc, ident)
            bias_exp = const.tile([B, 1], DT.float32)
            nc.vector.memset(bias_exp, -10.0)
            ones = const.tile([P, 1], DT.bfloat16)
            nc.vector.memset(ones, 1.0)
            eps12 = const.tile([P, 1], DT.float32)
            nc.vector.memset(eps12, 1e-12)
            ln10b = const.tile([P, 1], DT.float32)
            nc.vector.memset(ln10b, LN10)
            if WSUM_MODE == "gpsimd":
                wsum4 = const.tile([P, 4, B], DT.float32)
                nc.vector.memset(wsum4, 0.0)

            # ---- query preprocessing: qn^T bf16 chunks [128d, 4c, 64b] ----
            q_sb = const.tile([B, D], DT.float32)
            nc.sync.dma_start(out=q_sb, in_=q_d[:, :])
            qsq = const.tile([B, D], DT.float32)
            ssq = const.tile([B, 1], DT.float32)
            nc.scalar.activation(qsq, q_sb, AF.Square, accum_out=ssq)
            lnq = const.tile([B, 1], DT.float32)
            nc.scalar.activation(lnq, ssq, AF.Ln, bias=eps12[:B])
            invq = const.tile([B, 1], DT.float32)
            nc.scalar.activation(invq, lnq, AF.Exp, scale=-0.5)
            qn = const.tile([B, D], DT.bfloat16)
            nc.vector.tensor_scalar_mul(out=qn, in0=q_sb, scalar1=invq)
            qnT = const.tile([P, 4, B], DT.bfloat16)
            qt_ps = ps_wt.tile([P, 4, B], DT.bfloat16, tag="wt")
            for c in range(4):
                nc.tensor.matmul(qt_ps[:, c, :], lhsT=qn[:, c * P:(c + 1) * P],
                                 rhs=ident[:B, :B], start=True, stop=True,
                                 is_transpose=True)
            nc.scalar.copy(qnT, qt_ps)

            acc_ps = ps_acc.tile([B, D], DT.float32)
            if WSUM_MODE == "pe":
                l_ps = ps_l.tile([B, 1], DT.float32, tag="l")

            def norm_op(eng, sq, ss_col, a_t):
                if eng == "v":
                    nc.vector.affine_mul_reduce(
                        out=sq, accum_out=ss_col, in0=a_t, in1=a_t,
                        scale=1.0, bias=0.0)
                else:
                    nc.scalar.activation(sq, a_t, AF.Square, accum_out=ss_col)

            copy_eng = {"v": nc.vector.tensor_copy,
                        "a": nc.scalar.copy,
                        "g": nc.gpsimd.tensor_copy}

            # ---- main streaming loop over quads ----
            for qd in range(nquads):
                last = qd == nquads - 1
                a_sl = slab_pool.tile([P, 4, D], DT.bfloat16)
                r0 = qd * QROWS
                if not last or p_real == P:
                    nc.gpsimd.dma_start(
                        out=a_sl,
                        in_=a_d[r0:r0 + QROWS, :].rearrange(
                            "(p t) d -> p t d", p=P))
                else:
                    # partition slices must be 32-aligned; the DMA below
                    # overwrites partitions [96, p_real)
                    p0 = (p_real // 32) * 32
                    nc.gpsimd.memset(a_sl[p0:], 0)
                    nc.gpsimd.dma_start(
                        out=a_sl[:p_real],
                        in_=a_d[r0:npc, :].rearrange(
                            "(p t) d -> p t d", p=p_real))

                # row norms -> inv = 10/||a||  [128, 4] fp32
                ss = small.tile([P, 4], DT.float32, tag="ss")
                for t in range(4):
                    sq = small.tile([P, D], DT.bfloat16, tag="sq")
                    norm_op(NORM_PAT[t], sq, ss[:, t:t + 1], a_sl[:, t, :])
                lns = small.tile([P, 4], DT.float32, tag="lns")
                nc.scalar.activation(lns, ss, AF.Ln, bias=eps12)
                inv = small.tile([P, 4], DT.float32, tag="inv")
                nc.scalar.activation(inv, lns, AF.Exp, scale=-0.5, bias=ln10b)

                # A^T chunks, pre-scaled by inv via diag matmul
                at_q = at_pool.tile([P, 4, 4, P], DT.bfloat16)  # [d, c, t, j]
                for t in range(4):
                    diag = small.tile([P, P], DT.bfloat16, tag="diag")
                    nc.vector.tensor_scalar_mul(
                        out=diag, in0=ident, scalar1=inv[:, t:t + 1])
                    at_ps = ps_at.tile([P, 4, P], DT.bfloat16)
                    for c in range(4):
                        # is_transpose relaxes the fp32-PSUM rule; the rhs
                        # still streams through the array, so this computes
                        # A_chunk.T @ diag(inv) = A^T pre-scaled by 10/||a||
                        nc.tensor.matmul(
                            at_ps[:, c, :],
                            lhsT=a_sl[:, t, c * P:(c + 1) * P],
                            rhs=diag, start=True, stop=True,
                            is_transpose=True)
                    copy_eng[ATCOPY_PAT[t]](at_q[:, :, t, :], at_ps)

                # sims: s[b, 512j] accumulated over 4 d-chunks
                s_ps = ps_s.tile([B, 4 * P], DT.float32, tag="s")
                for c in range(4):
                    nc.tensor.matmul(
                        s_ps, lhsT=qnT[:, c, :], rhs=at_q[:, c, :, :],
                        start=(c == 0), stop=(c == 3))

                # w = exp(s - 10)   [64, 512] bf16
                w_q = wq_pool.tile([B, 4 * P], DT.bfloat16)
                nc.scalar.activation(w_q, s_ps, AF.Exp, bias=bias_exp)

                # w^T [128j, 4t, 64b]
                wt_ps = ps_wt.tile([P, 4, B], DT.bfloat16, tag="wt")
                for t in range(4):
                    nc.tensor.matmul(
                        wt_ps[:, t, :], lhsT=w_q[:, t * P:(t + 1) * P],
                        rhs=ident[:B, :B], start=True, stop=True,
                        is_transpose=True)
                wt_sb = wt_pool.tile([P, 4, B], DT.bfloat16)
                copy_eng[WTCOPY_ENG](wt_sb, wt_ps)

                # retrieval: acc += w^T.T @ A ; lsum += w^T.T @ 1
                for t in range(4):
                    gt = 4 * qd + t
                    nc.tensor.matmul(
                        acc_ps, lhsT=wt_sb[:, t, :], rhs=a_sl[:, t, :],
                        start=(gt == 0), stop=(gt == ntiles - 1))
                    if WSUM_MODE == "pe":
                        nc.tensor.matmul(
                            l_ps, lhsT=wt_sb[:, t, :], rhs=ones,
                            start=(gt == 0), stop=(gt == ntiles - 1))
                if WSUM_MODE == "gpsimd":
                    nc.gpsimd.tensor_add(wsum4, wsum4, wt_sb)

            # ---- epilogue: normalizer + writeback ----
            if WSUM_MODE == "gpsimd":
                l_ps = ps_l.tile([B, 1], DT.float32, tag="l")
                for t in range(4):
                    nc.tensor.matmul(l_ps, lhsT=wsum4[:, t, :], rhs=ones,
                                     start=(t == 0), stop=(t == 3))
            acc_sb = const.tile([B, D], DT.float32)
            nc.vector.tensor_copy(acc_sb, acc_ps)
            l_sb = const.tile([B, 1], DT.float32)
            nc.vector.tensor_copy(l_sb, l_ps)
            nc.sync.dma_start(out=acc_d[:, :], in_=acc_sb)
            nc.sync.dma_start(out=lsum_d[:, :], in_=l_sb)

    nc.finalize()
    return nc


_NC_CACHE = {}


def _get_nc(npc=NPC):
    if npc not in _NC_CACHE:
        _NC_CACHE[npc] = _build(npc)
    return _NC_CACHE[npc]


def kernel(query, addresses):
    global LAST_RESULTS
    query = np.ascontiguousarray(np.asarray(query), dtype=np.float32)
    addresses = np.ascontiguousarray(np.asarray(addresses), dtype=np.float32)
    n = addresses.shape[0]
    npc = n // NCORES
    assert npc * NCORES == n
    nc = _get_nc(npc)
    in_maps = [
        {"query": query, "addresses": addresses[c * npc:(c + 1) * npc]}
        for c in range(NCORES)
    ]
    res = run_bass_kernel_spmd(nc, in_maps, core_ids=list(range(NCORES)))
    LAST_RESULTS = res
    acc = np.zeros((B, D), np.float64)
    l = np.zeros((B, 1), np.float64)
    nquads = (npc + QROWS - 1) // QROWS
    n_pad = nquads * QROWS - npc  # zero rows in the padded last quad
    for r in res.results:
        acc += r["acc"].astype(np.float64)
        l += r["lsum"].astype(np.float64)
        if n_pad:
            # each pad row contributes exactly exp(0 - 10)
            l -= n_pad * math.exp(-10.0)
    return (acc / l).astype(np.float32)


# revision 13
# speedup vs baseline: 1.5907x; 1.4800x over previous
"""Trainium2 Bass kernel: cosine-similarity softmin retrieval (DSDM).

reference:  qn = q/||q||; an = a/||a||; sims = qn @ an^T            [B, N]
            w = softmax(10*sims) over N  (softmin of (1-sims)/0.1)
            out = (w @ A)                                           [B, D]

v3 strategy (8 NeuronCores, flash-attention-style split over N):
  - addresses [200000, 512] sharded row-wise, 25000 rows/core.
  - per core the shard streams once in 512-row "quads" (49 of them):
      * one SWDGE cast-DMA per quad, row-permuted (p t) d -> p t d:
        partition p holds rows 4p..4p+3 => 8KB contiguous descriptors.
        The permutation cancels between sims and retrieval.
      * row norms ss = sum(a^2): DVE affine_mul_reduce / ACT Square
        per NORM_PAT; inv = 10/||a|| = exp(-0.5*ln(ss+eps)+ln10) (ACT)
      * A^T via PE transposes (2 tiles per PSUM bank -> one DVE copy
        per 2 tiles)
      * sims computed TRANSPOSED: s^T[j, b] with lhsT = A^T chunks,
        rhs = qn^T (j is the partition dim, so the per-row 10/||a||
        scale rides the ACT exp's per-partition scale operand for free)
      * w^T = Exp(s^T * inv - 10) per tile on ACT -> [128, 4, 64] bf16
        (fixed shift: cos<=1 so logit-10 <= 0; no running max needed)
      * retrieval: acc[64, 512] += w^T.T @ A in PSUM across all tiles
      * lsum: wsum[128, 4, 64] += w^T on GPSIMD; ones-matmul at end
  - host: out = sum_c acc_c / sum_c l_c   (gather/unshard + tiny divide)

Padding: per-core 25000 rows = 48 full quads + 424 rows (partitions
0..105 of quad 48); partitions 106..127 are zeroed and contribute
exactly exp(-10) each to lsum, subtracted on the host.
"""

import math
import os
from collections import OrderedDict

import numpy as np

import concourse.bass as bass
import concourse.tile as tile
from concourse import bacc, mybir
from concourse.bass_utils import run_bass_kernel_spmd
from concourse.masks import make_identity

DT = mybir.dt
AF = mybir.ActivationFunctionType

B = 64
D = 512
N_FULL = 200000
NCORES = 8
NPC = N_FULL // NCORES  # 25000
P = 128
QROWS = 4 * P  # rows per quad
LN10 = math.log(10.0)

# engine assignment knobs (v=DVE, a=ACT), one char per tile-in-quad
NORM_PAT = os.environ.get("KERNEL_NORM_PAT", "vvaa")
WSUM_MODE = os.environ.get("KERNEL_WSUM", "gpsimd")  # "pe" or "gpsimd"
ABUFS = int(os.environ.get("KERNEL_ABUFS", "8"))
ATBUFS = int(os.environ.get("KERNEL_ATBUFS", "4"))
LDW_OPT = os.environ.get("KERNEL_LDW_OPT", "0") == "1"

LAST_RESULTS = None  # test harness reads exec_time_ns from here


def _patch_act_tables():
    """Prefer the combined natural_log_exp set so Ln/Exp/Square/Copy share
    one ACT table load instead of thrashing 2 loads per quad (~2.7us each)."""
    if getattr(bacc.get_activation_tables, "_patched", False):
        return
    orig = bacc.get_activation_tables

    keep = {AF.Ln, AF.Exp, AF.Square, AF.Copy}

    def patched(arch):
        tabs = orig(arch)
        out = OrderedDict()
        for k, fns in tabs.items():
            if k == "natural_log_exp_and_others":
                out[k] = fns
            else:
                out[k] = {f for f in fns if f not in keep}
        return out

    patched._patched = True
    bacc.get_activation_tables = patched


def _patch_ldw_opt():
    """Opt-in: flip walrus --enable-ldw-opt to true (experiment knob)."""
    from concourse import bass_utils

    if getattr(bass_utils.run_command, "_ldw_patched", False):
        return
    orig = bass_utils.run_command

    def patched(cmd, *a, **kw):
        cmd = [c.replace("--enable-ldw-opt=false", "--enable-ldw-opt=true")
               if isinstance(c, str) else c for c in cmd]
        return orig(cmd, *a, **kw)

    patched._ldw_patched = True
    bass_utils.run_command = patched


def _build(npc=NPC):
    _patch_act_tables()
    if LDW_OPT:
        _patch_ldw_opt()
    assert npc % 4 == 0
    nquads = (npc + QROWS - 1) // QROWS
    ntiles = 4 * nquads
    r_last = npc - (nquads - 1) * QROWS
    assert r_last % 4 == 0
    p_real = r_last // 4  # 106 for npc=25000

    nc = bacc.Bacc("TRN2")
    q_d = nc.dram_tensor("query", [B, D], DT.float32, kind="ExternalInput")
    a_d = nc.dram_tensor("addresses", [npc, D], DT.float32, kind="ExternalInput")
    acc_d = nc.dram_tensor("acc", [B, D], DT.float32, kind="ExternalOutput")
    lsum_d = nc.dram_tensor("lsum", [B, 1], DT.float32, kind="ExternalOutput")

    with tile.TileContext(nc) as tc:
        with (
            tc.tile_pool(name="const", bufs=1) as const,
            tc.tile_pool(name="slab", bufs=ABUFS) as slab_pool,
            tc.tile_pool(name="at", bufs=ATBUFS) as at_pool,
            tc.tile_pool(name="wt", bufs=3) as wt_pool,
            tc.tile_pool(name="small", bufs=4) as small,
            tc.tile_pool(name="ps_at", bufs=4, space="PSUM") as ps_at,
            tc.tile_pool(name="ps_s", bufs=2, space="PSUM") as ps_s,
            tc.tile_pool(name="ps_acc", bufs=1, space="PSUM") as ps_acc,
            tc.tile_pool(name="ps_l", bufs=1, space="PSUM") as ps_l,
        ):
            ident = const.tile([P, P], DT.bfloat16)
            make_identity(nc, ident)
            bias_exp = const.tile([P, 1], DT.float32)
            nc.vector.memset(bias_exp, -10.0)
            ones = const.tile([P, 1], DT.bfloat16)
            nc.vector.memset(ones, 1.0)
            onesf = const.tile([P, 1], DT.float32)
            nc.vector.memset(onesf, 1.0)
            eps12 = const.tile([P, 1], DT.float32)
            nc.vector.memset(eps12, 1e-12)
            ln10b = const.tile([P, 1], DT.float32)
            nc.vector.memset(ln10b, LN10)
            wsum4 = const.tile([P, 4, B], DT.float32)
            nc.vector.memset(wsum4, 0.0)

            # ---- query preprocessing: qn^T bf16 chunks [128d, 4c, 64b] ----
            q_sb = const.tile([B, D], DT.float32)
            nc.sync.dma_start(out=q_sb, in_=q_d[:, :])
            qsq = const.tile([B, D], DT.float32)
            ssq = const.tile([B, 1], DT.float32)
            nc.scalar.activation(qsq, q_sb, AF.Square, accum_out=ssq)
            lnq = const.tile([B, 1], DT.float32)
            nc.scalar.activation(lnq, ssq, AF.Ln, bias=eps12[:B])
            invq = const.tile([B, 1], DT.float32)
            nc.scalar.activation(invq, lnq, AF.Exp, scale=-0.5)
            qn = const.tile([B, D], DT.bfloat16)
            nc.vector.tensor_scalar_mul(out=qn, in0=q_sb, scalar1=invq)
            qnT = const.tile([P, 4, B], DT.bfloat16)
            qt_ps = ps_l.tile([P, 4, B], DT.bfloat16, tag="l")
            for c in range(4):
                nc.tensor.transpose(qt_ps[:, c, :], qn[:, c * P:(c + 1) * P],
                                    ident[:B, :B])
            nc.scalar.copy(qnT, qt_ps)

            acc_ps = ps_acc.tile([B, D], DT.float32)
            if WSUM_MODE == "pe":
                l_ps = ps_l.tile([B, 1], DT.float32, tag="l")

            def norm_op(eng, sq, ss_col, a_t):
                if eng == "v":
                    nc.vector.affine_mul_reduce(
                        out=sq, accum_out=ss_col, in0=a_t, in1=a_t,
                        scale=1.0, bias=0.0)
                else:
                    nc.scalar.activation(sq, a_t, AF.Square, accum_out=ss_col)

            # ---- main streaming loop over quads ----
            for qd in range(nquads):
                last = qd == nquads - 1
                a_sl = slab_pool.tile([P, 4, D], DT.bfloat16)
                r0 = qd * QROWS
                if not last or p_real == P:
                    nc.gpsimd.dma_start(
                        out=a_sl,
                        in_=a_d[r0:r0 + QROWS, :].rearrange(
                            "(p t) d -> p t d", p=P))
                else:
                    # partition slices must be 32-aligned; the DMA below
                    # overwrites partitions [p0, p_real)
                    p0 = (p_real // 32) * 32
                    nc.gpsimd.memset(a_sl[p0:], 0)
                    nc.gpsimd.dma_start(
                        out=a_sl[:p_real],
                        in_=a_d[r0:npc, :].rearrange(
                            "(p t) d -> p t d", p=p_real))

                # row norms -> inv = 10/||a||  [128, 4] fp32
                ss = small.tile([P, 4], DT.float32, tag="ss")
                for t in range(4):
                    sq = small.tile([P, D], DT.bfloat16, tag="sq")
                    norm_op(NORM_PAT[t], sq, ss[:, t:t + 1], a_sl[:, t, :])
                lns = small.tile([P, 4], DT.float32, tag="lns")
                nc.scalar.activation(lns, ss, AF.Ln, bias=eps12)
                inv = small.tile([P, 4], DT.float32, tag="inv")
                nc.scalar.activation(inv, lns, AF.Exp, scale=-0.5, bias=ln10b)

                # A^T chunks: [d, tt, c, j], two tiles per PSUM bank
                at_sb = []
                for pair in range(2):
                    at_ps2 = ps_at.tile([P, 2, 4, P], DT.bfloat16)
                    for tt in range(2):
                        t = 2 * pair + tt
                        for c in range(4):
                            nc.tensor.transpose(
                                at_ps2[:, tt, c, :],
                                a_sl[:, t, c * P:(c + 1) * P], ident)
                    at_sb2 = at_pool.tile([P, 2, 4, P], DT.bfloat16)
                    nc.vector.tensor_copy(at_sb2, at_ps2)
                    at_sb.append(at_sb2)

                # sims transposed: s^T[j, b] accumulated over 4 d-chunks
                s_q = ps_s.tile([P, 4, B], DT.float32, tag="s")
                for t in range(4):
                    pair, tt = divmod(t, 2)
                    for c in range(4):
                        nc.tensor.matmul(
                            s_q[:, t, :], lhsT=at_sb[pair][:, tt, c, :],
                            rhs=qnT[:, c, :], start=(c == 0), stop=(c == 3))

                # w^T = exp(s^T * inv - 10)   [128, 4, 64] bf16
                wt_q = wt_pool.tile([P, 4, B], DT.bfloat16)
                for t in range(4):
                    nc.scalar.activation(
                        wt_q[:, t, :], s_q[:, t, :], AF.Exp,
                        bias=bias_exp, scale=inv[:, t:t + 1])

                # retrieval: acc += w^T.T @ A ; lsum via wsum accumulation
                for t in range(4):
                    gt = 4 * qd + t
                    nc.tensor.matmul(
                        acc_ps, lhsT=wt_q[:, t, :], rhs=a_sl[:, t, :],
                        start=(gt == 0), stop=(gt == ntiles - 1))
                    if WSUM_MODE == "pe":
                        nc.tensor.matmul(
                            l_ps, lhsT=wt_q[:, t, :], rhs=ones,
                            start=(gt == 0), stop=(gt == ntiles - 1))
                if WSUM_MODE == "gpsimd":
                    nc.gpsimd.tensor_add(wsum4, wsum4, wt_q)

            # ---- epilogue: normalizer + writeback ----
            if WSUM_MODE == "gpsimd":
                l_ps = ps_l.tile([B, 1], DT.float32, tag="l")
                for t in range(4):
                    nc.tensor.matmul(l_ps, lhsT=wsum4[:, t, :], rhs=onesf,
                                     start=(t == 0), stop=(t == 3))
            acc_sb = const.tile([B, D], DT.float32)
            nc.vector.tensor_copy(acc_sb, acc_ps)
            l_sb = const.tile([B, 1], DT.float32)
            nc.vector.tensor_copy(l_sb, l_ps)
            nc.sync.dma_start(out=acc_d[:, :], in_=acc_sb)
            nc.sync.dma_start(out=lsum_d[:, :], in_=l_sb)

    nc.finalize()
    return nc


_NC_CACHE = {}


def _get_nc(npc=NPC):
    if npc not in _NC_CACHE:
        _NC_CACHE[npc] = _build(npc)
    return _NC_CACHE[npc]


def kernel(query, addresses):
    global LAST_RESULTS
    query = np.ascontiguousarray(np.asarray(query), dtype=np.float32)
    addresses = np.ascontiguousarray(np.asarray(addresses), dtype=np.float32)
    n = addresses.shape[0]
    npc = n // NCORES
    assert npc * NCORES == n
    nc = _get_nc(npc)
    in_maps = [
        {"query": query, "addresses": addresses[c * npc:(c + 1) * npc]}
        for c in range(NCORES)
    ]
    res = run_bass_kernel_spmd(nc, in_maps, core_ids=list(range(NCORES)))
    LAST_RESULTS = res
    acc = np.zeros((B, D), np.float64)
    l = np.zeros((B, 1), np.float64)
    nquads = (npc + QROWS - 1) // QROWS
    n_pad = nquads * QROWS - npc  # zero rows in the padded last quad
    for r in res.results:
        acc += r["acc"].astype(np.float64)
        l += r["lsum"].astype(np.float64)
        if n_pad:
            # each pad row contributes exactly exp(0 - 10)
            l -= n_pad * math.exp(-10.0)
    return (acc / l).astype(np.float32)


# revision 16
# speedup vs baseline: 1.6487x; 1.0365x over previous
"""Trainium2 Bass kernel: cosine-similarity softmin retrieval (DSDM).

reference:  qn = q/||q||; an = a/||a||; sims = qn @ an^T            [B, N]
            w = softmax(10*sims) over N  (softmin of (1-sims)/0.1)
            out = (w @ A)                                           [B, D]

v3 strategy (8 NeuronCores, flash-attention-style split over N):
  - addresses [200000, 512] sharded row-wise, 25000 rows/core.
  - per core the shard streams once in 512-row "quads" (49 of them):
      * one SWDGE cast-DMA per quad, row-permuted (p t) d -> p t d:
        partition p holds rows 4p..4p+3 => 8KB contiguous descriptors.
        The permutation cancels between sims and retrieval.
      * row norms ss = sum(a^2): DVE affine_mul_reduce / ACT Square
        per NORM_PAT; inv = 10/||a|| = exp(-0.5*ln(ss+eps)+ln10) (ACT)
      * A^T via PE transposes (2 tiles per PSUM bank -> one DVE copy
        per 2 tiles)
      * sims computed TRANSPOSED: s^T[j, b] with lhsT = A^T chunks,
        rhs = qn^T (j is the partition dim, so the per-row 10/||a||
        scale rides the ACT exp's per-partition scale operand for free)
      * w^T = Exp(s^T * inv - 10) per tile on ACT -> [128, 4, 64] bf16
        (fixed shift: cos<=1 so logit-10 <= 0; no running max needed)
      * retrieval: acc[64, 512] += w^T.T @ A in PSUM across all tiles
      * lsum: wsum[128, 4, 64] += w^T on GPSIMD; ones-matmul at end
  - host: out = sum_c acc_c / sum_c l_c   (gather/unshard + tiny divide)

Padding: per-core 25000 rows = 48 full quads + 424 rows (partitions
0..105 of quad 48); partitions 106..127 are zeroed and contribute
exactly exp(-10) each to lsum, subtracted on the host.
"""

import math
import os
from collections import OrderedDict

import numpy as np

import concourse.bass as bass
import concourse.tile as tile
from concourse import bacc, mybir
from concourse.bass_utils import run_bass_kernel_spmd
from concourse.masks import make_identity

DT = mybir.dt
AF = mybir.ActivationFunctionType

B = 64
D = 512
N_FULL = 200000
NCORES = 8
NPC = N_FULL // NCORES  # 25000
P = 128
QROWS = 4 * P  # rows per quad
LN10 = math.log(10.0)

# engine assignment knobs (v=DVE, a=ACT), one char per tile-in-quad
NORM_PAT = os.environ.get("KERNEL_NORM_PAT", "vvvv")
DMA_AHEAD = int(os.environ.get("KERNEL_DMA_AHEAD", "3"))
WSUM_MODE = os.environ.get("KERNEL_WSUM", "gpsimd")  # "pe" or "gpsimd"
ABUFS = int(os.environ.get("KERNEL_ABUFS", "8"))
ATBUFS = int(os.environ.get("KERNEL_ATBUFS", "4"))
LDW_OPT = os.environ.get("KERNEL_LDW_OPT", "0") == "1"

LAST_RESULTS = None  # test harness reads exec_time_ns from here


def _patch_act_tables():
    """Prefer the combined natural_log_exp set so Ln/Exp/Square/Copy share
    one ACT table load instead of thrashing 2 loads per quad (~2.7us each)."""
    if getattr(bacc.get_activation_tables, "_patched", False):
        return
    orig = bacc.get_activation_tables

    keep = {AF.Ln, AF.Exp, AF.Square, AF.Copy}

    def patched(arch):
        tabs = orig(arch)
        out = OrderedDict()
        for k, fns in tabs.items():
            if k == "natural_log_exp_and_others":
                out[k] = fns
            else:
                out[k] = {f for f in fns if f not in keep}
        return out

    patched._patched = True
    bacc.get_activation_tables = patched


def _patch_ldw_opt():
    """Opt-in: flip walrus --enable-ldw-opt to true (experiment knob)."""
    from concourse import bass_utils

    if getattr(bass_utils.run_command, "_ldw_patched", False):
        return
    orig = bass_utils.run_command

    def patched(cmd, *a, **kw):
        cmd = [c.replace("--enable-ldw-opt=false", "--enable-ldw-opt=true")
               if isinstance(c, str) else c for c in cmd]
        return orig(cmd, *a, **kw)

    patched._ldw_patched = True
    bass_utils.run_command = patched


def _build(npc=NPC):
    _patch_act_tables()
    if LDW_OPT:
        _patch_ldw_opt()
    assert npc % 4 == 0
    nquads = (npc + QROWS - 1) // QROWS
    ntiles = 4 * nquads
    r_last = npc - (nquads - 1) * QROWS
    assert r_last % 4 == 0
    p_real = r_last // 4  # 106 for npc=25000

    nc = bacc.Bacc("TRN2")
    q_d = nc.dram_tensor("query", [B, D], DT.float32, kind="ExternalInput")
    a_d = nc.dram_tensor("addresses", [npc, D], DT.float32, kind="ExternalInput")
    acc_d = nc.dram_tensor("acc", [B, D], DT.float32, kind="ExternalOutput")
    lsum_d = nc.dram_tensor("lsum", [B, 1], DT.float32, kind="ExternalOutput")

    with tile.TileContext(nc) as tc:
        with (
            tc.tile_pool(name="const", bufs=1) as const,
            tc.tile_pool(name="slab", bufs=ABUFS) as slab_pool,
            tc.tile_pool(name="at", bufs=ATBUFS) as at_pool,
            tc.tile_pool(name="wt", bufs=3) as wt_pool,
            tc.tile_pool(name="small", bufs=4) as small,
            tc.tile_pool(name="ps_at", bufs=4, space="PSUM") as ps_at,
            tc.tile_pool(name="ps_s", bufs=2, space="PSUM") as ps_s,
            tc.tile_pool(name="ps_acc", bufs=1, space="PSUM") as ps_acc,
            tc.tile_pool(name="ps_l", bufs=1, space="PSUM") as ps_l,
        ):
            ident = const.tile([P, P], DT.bfloat16)
            make_identity(nc, ident)
            bias_exp = const.tile([P, 1], DT.float32)
            nc.vector.memset(bias_exp, -10.0)
            ones = const.tile([P, 1], DT.bfloat16)
            nc.vector.memset(ones, 1.0)
            onesf = const.tile([P, 1], DT.float32)
            nc.vector.memset(onesf, 1.0)
            eps12 = const.tile([P, 1], DT.float32)
            nc.vector.memset(eps12, 1e-12)
            ln10b = const.tile([P, 1], DT.float32)
            nc.vector.memset(ln10b, LN10)
            wsum4 = const.tile([P, 4, B], DT.float32)
            nc.vector.memset(wsum4, 0.0)

            # ---- query preprocessing: qn^T bf16 chunks [128d, 4c, 64b] ----
            q_sb = const.tile([B, D], DT.float32)
            nc.sync.dma_start(out=q_sb, in_=q_d[:, :])
            qsq = const.tile([B, D], DT.float32)
            ssq = const.tile([B, 1], DT.float32)
            nc.scalar.activation(qsq, q_sb, AF.Square, accum_out=ssq)
            lnq = const.tile([B, 1], DT.float32)
            nc.scalar.activation(lnq, ssq, AF.Ln, bias=eps12[:B])
            invq = const.tile([B, 1], DT.float32)
            nc.scalar.activation(invq, lnq, AF.Exp, scale=-0.5)
            qn = const.tile([B, D], DT.bfloat16)
            nc.vector.tensor_scalar_mul(out=qn, in0=q_sb, scalar1=invq)
            qnT = const.tile([P, 4, B], DT.bfloat16)
            qt_ps = ps_l.tile([P, 4, B], DT.bfloat16, tag="l")
            for c in range(4):
                nc.tensor.transpose(qt_ps[:, c, :], qn[:, c * P:(c + 1) * P],
                                    ident[:B, :B])
            nc.scalar.copy(qnT, qt_ps)

            acc_ps = ps_acc.tile([B, D], DT.float32)
            if WSUM_MODE == "pe":
                l_ps = ps_l.tile([B, 1], DT.float32, tag="l")

            def norm_op(eng, sq, ss_col, a_t):
                if eng == "v":
                    nc.vector.affine_mul_reduce(
                        out=sq, accum_out=ss_col, in0=a_t, in1=a_t,
                        scale=1.0, bias=0.0)
                else:
                    nc.scalar.activation(sq, a_t, AF.Square, accum_out=ss_col)

            # ---- main streaming loop over quads ----
            # loads are emitted DMA_AHEAD quads early so the SWDGE issue
            # (gpsimd queue) isn't gated behind the same quad's wsum add
            a_slabs = {}

            def emit_load(qd):
                last = qd == nquads - 1
                a_sl = slab_pool.tile([P, 4, D], DT.bfloat16)
                r0 = qd * QROWS
                if not last or p_real == P:
                    nc.gpsimd.dma_start(
                        out=a_sl,
                        in_=a_d[r0:r0 + QROWS, :].rearrange(
                            "(p t) d -> p t d", p=P))
                else:
                    # partition slices must be 32-aligned; the DMA below
                    # overwrites partitions [p0, p_real)
                    p0 = (p_real // 32) * 32
                    nc.gpsimd.memset(a_sl[p0:], 0)
                    nc.gpsimd.dma_start(
                        out=a_sl[:p_real],
                        in_=a_d[r0:npc, :].rearrange(
                            "(p t) d -> p t d", p=p_real))
                a_slabs[qd] = a_sl

            for qd in range(min(DMA_AHEAD + 1, nquads)):
                emit_load(qd)
            for qd in range(nquads):
                a_sl = a_slabs.pop(qd)

                # row norms -> inv = 10/||a||  [128, 4] fp32
                ss = small.tile([P, 4], DT.float32, tag="ss")
                for t in range(4):
                    sq = small.tile([P, D], DT.bfloat16, tag="sq")
                    norm_op(NORM_PAT[t], sq, ss[:, t:t + 1], a_sl[:, t, :])
                lns = small.tile([P, 4], DT.float32, tag="lns")
                nc.scalar.activation(lns, ss, AF.Ln, bias=eps12)
                inv = small.tile([P, 4], DT.float32, tag="inv")
                nc.scalar.activation(inv, lns, AF.Exp, scale=-0.5, bias=ln10b)

                # A^T chunks: [d, tt, c, j], two tiles per PSUM bank
                at_sb = []
                for pair in range(2):
                    at_ps2 = ps_at.tile([P, 2, 4, P], DT.bfloat16)
                    for tt in range(2):
                        t = 2 * pair + tt
                        for c in range(4):
                            nc.tensor.transpose(
                                at_ps2[:, tt, c, :],
                                a_sl[:, t, c * P:(c + 1) * P], ident)
                    at_sb2 = at_pool.tile([P, 2, 4, P], DT.bfloat16)
                    nc.vector.tensor_copy(at_sb2, at_ps2)
                    at_sb.append(at_sb2)

                # sims transposed: s^T[j, b] accumulated over 4 d-chunks
                s_q = ps_s.tile([P, 4, B], DT.float32, tag="s")
                for t in range(4):
                    pair, tt = divmod(t, 2)
                    for c in range(4):
                        nc.tensor.matmul(
                            s_q[:, t, :], lhsT=at_sb[pair][:, tt, c, :],
                            rhs=qnT[:, c, :], start=(c == 0), stop=(c == 3))

                # w^T = exp(s^T * inv - 10)   [128, 4, 64] bf16
                wt_q = wt_pool.tile([P, 4, B], DT.bfloat16)
                for t in range(4):
                    nc.scalar.activation(
                        wt_q[:, t, :], s_q[:, t, :], AF.Exp,
                        bias=bias_exp, scale=inv[:, t:t + 1])

                # retrieval: acc += w^T.T @ A ; lsum via wsum accumulation
                for t in range(4):
                    gt = 4 * qd + t
                    nc.tensor.matmul(
                        acc_ps, lhsT=wt_q[:, t, :], rhs=a_sl[:, t, :],
                        start=(gt == 0), stop=(gt == ntiles - 1))
                    if WSUM_MODE == "pe":
                        nc.tensor.matmul(
                            l_ps, lhsT=wt_q[:, t, :], rhs=ones,
                            start=(gt == 0), stop=(gt == ntiles - 1))
                if WSUM_MODE == "gpsimd":
                    nc.gpsimd.tensor_add(wsum4, wsum4, wt_q)
                if qd + DMA_AHEAD + 1 < nquads:
                    emit_load(qd + DMA_AHEAD + 1)

            # ---- epilogue: normalizer + writeback ----
            if WSUM_MODE == "gpsimd":
                l_ps = ps_l.tile([B, 1], DT.float32, tag="l")
                for t in range(4):
                    nc.tensor.matmul(l_ps, lhsT=wsum4[:, t, :], rhs=onesf,
                                     start=(t == 0), stop=(t == 3))
            acc_sb = const.tile([B, D], DT.float32)
            nc.vector.tensor_copy(acc_sb, acc_ps)
            l_sb = const.tile([B, 1], DT.float32)
            nc.vector.tensor_copy(l_sb, l_ps)
            nc.sync.dma_start(out=acc_d[:, :], in_=acc_sb)
            nc.sync.dma_start(out=lsum_d[:, :], in_=l_sb)

    nc.finalize()
    return nc


_NC_CACHE = {}


def _get_nc(npc=NPC):
    if npc not in _NC_CACHE:
        _NC_CACHE[npc] = _build(npc)
    return _NC_CACHE[npc]


def kernel(query, addresses):
    global LAST_RESULTS
    query = np.ascontiguousarray(np.asarray(query), dtype=np.float32)
    addresses = np.ascontiguousarray(np.asarray(addresses), dtype=np.float32)
    n = addresses.shape[0]
    npc = n // NCORES
    assert npc * NCORES == n
    nc = _get_nc(npc)
    in_maps = [
        {"query": query, "addresses": addresses[c * npc:(c + 1) * npc]}
        for c in range(NCORES)
    ]
    res = run_bass_kernel_spmd(nc, in_maps, core_ids=list(range(NCORES)))
    LAST_RESULTS = res
    acc = np.zeros((B, D), np.float64)
    l = np.zeros((B, 1), np.float64)
    nquads = (npc + QROWS - 1) // QROWS
    n_pad = nquads * QROWS - npc  # zero rows in the padded last quad
    for r in res.results:
        acc += r["acc"].astype(np.float64)
        l += r["lsum"].astype(np.float64)
        if n_pad:
            # each pad row contributes exactly exp(0 - 10)
            l -= n_pad * math.exp(-10.0)
    return (acc / l).astype(np.float32)
